# revision 25
# baseline (speedup 1.0000x reference)
"""Trainium2 Bass kernel: 2D valid cross-correlation (4096x4096 image, 15x15 kernel).

Strategy: shard output COLUMNS across 8 NeuronCores (spatial data-parallel,
14-column halo overlap in the input slices; no device-to-device
communication). Each core computes all 4082 output rows for its 512 output
columns as 15 PSUM-accumulated bf16 matmuls per 114-row tile: for kernel
column b, the stationary operand is the 128x114 banded Toeplitz matrix
T_b[r, m] = w[r - m, b] (contraction over 128 input rows -> 114 output
rows) and the moving operand is the natural row-major X tile offset by b
columns in the free dimension. PSUM accumulation is fp32; the result is
stored as bf16 and widened to fp32 on the host.

bf16 weight loads use the hardware fast-weight-load path and pipeline under
the in-flight matmul stream (the PE pulls LDWEIGHTS ahead), so the 15
per-tile weight reloads are free and the stream runs at the 512-cycle
(~216ns) per-matmul floor. bf16 also halves input DMA traffic. X is padded
to uniform 128-row tiles on the host; input prefetch is 18 tiles deep and
PSUM banks rotate 8-deep. All input/output DMAs ride the sync HWDGE queue
and all PSUM->SBUF drains ride vector, leaving gpsimd/scalar idle (a gpsimd
SWDGE backlog costs ~5us of teardown drain; out-DMAs on the drain engine's
queue delay PSUM bank recycling). Dummy matmuls fill the ~3us window where
the first weight slice's DMA receipt gates the real stream, so the PE
clock-gate (1.2GHz cold / 2.4GHz warm) is already open when it starts.

Accuracy: inputs and the stored output are rounded to bf16 (~3e-3 relative
error combined vs the fp32 reference; the harness gate is 2e-2). PSUM
accumulation itself is exact fp32.
"""

import numpy as np

import bass_rust
import concourse.bass as bass
import concourse.mybir as mybir
import concourse.tile as tile
from concourse import bacc
from concourse.bass_utils import run_bass_kernel_spmd

H, W = 4096, 4096
KH, KW = 15, 15
OH, OW = H - KH + 1, W - KW + 1  # 4082 x 4082

NCORES = 8
COLS_PER_CORE = 512               # output cols per core (core 7: 498 valid)
IN_COLS = COLS_PER_CORE + KW - 1  # 526 input cols (with halo)

MT = 114                          # output rows per tile (K = MT + 14 = 128)
NT = COLS_PER_CORE                # 512 output cols = one fp32 PSUM bank
NTILES = 36                       # ceil(4082 / 114); 36*114 = 4104 rows
H_PAD = NTILES * MT + KH - 1      # 4118 padded input rows (uniform K=128)

F32 = mybir.dt.float32
BF16 = mybir.dt.bfloat16


def _build_program():
    nc = bacc.Bacc("TRN2", target_bir_lowering=False, debug=False)
    x = nc.dram_tensor("x", [H_PAD, IN_COLS], BF16, kind="ExternalInput").ap()
    wt = nc.dram_tensor("wt", [128, KW * MT], BF16, kind="ExternalInput").ap()
    out = nc.dram_tensor("out", [NTILES * MT, NT], BF16, kind="ExternalOutput").ap()

    with tile.TileContext(nc) as tc:
        with (
            tc.tile_pool(name="wpool", bufs=1) as wpool,
            tc.tile_pool(name="xpool", bufs=18) as xpool,
            tc.tile_pool(name="opool", bufs=8) as opool,
            tc.tile_pool(name="ppool", bufs=8, space="PSUM") as ppool,
        ):
            # Toeplitz pack, split into per-slice DMAs so the completion
            # receipts (~2.5us each) for the first kernel-column slices land
            # before the matmul stream reaches them. wt0 rides the otherwise
            # empty scalar queue: its receipt gates the first real matmul,
            # and on a shared queue it fires ~1.2us later.
            wt0 = wpool.tile([128, MT], BF16, tag="wt0")
            nc.scalar.dma_start(wt0[:], wt[:, :MT])
            wt12 = wpool.tile([128, 2 * MT], BF16, tag="wt12")
            nc.sync.dma_start(wt12[:], wt[:, MT : 3 * MT])
            wt3 = wpool.tile([128, MT], BF16, tag="wt3")
            nc.sync.dma_start(wt3[:], wt[:, 3 * MT : 4 * MT])
            wtrest = wpool.tile([128, (KW - 4) * MT], BF16, tag="wtr")

            def wslice(b):
                if b == 0:
                    return wt0[:, :]
                if b in (1, 2):
                    return wt12[:, (b - 1) * MT : b * MT]
                if b == 3:
                    return wt3[:, :]
                return wtrest[:, (b - 4) * MT : (b - 3) * MT]

            prev_mm = None

            # HAM pre-warm: the first real matmul is gated on the wt0 DMA
            # completion semaphore (~2.5us of receipt latency); run dummy
            # matmuls in that window so the PE clock-gate (4/8 when cold)
            # opens closer to the start of the real stream.
            dz = wpool.tile([128, NT], BF16, tag="dz")
            nc.vector.memset(dz[:], 0)
            dacc = ppool.tile([MT, NT], F32, tag="acc", name="dacc")
            for _ in range(7):
                mm = nc.tensor.matmul(
                    dacc[:], dz[:, :MT], dz[:], start=True, stop=True
                )
                if prev_mm is not None:
                    bass_rust.add_dep_helper(
                        mm.ins, prev_mm.ins, False, "pe stream order"
                    )
                prev_mm = mm
            b0_mms = []
            for t in range(NTILES):
                xt = xpool.tile([128, IN_COLS], BF16, tag="xt", name=f"xt{t}")
                xdma = nc.sync.dma_start(xt[:], x[t * MT : t * MT + 128, :])
                # Throttle the input prefetch: with all 18 pool slots free at
                # t=0, the sync engine would fire an 18-tile burst that
                # saturates the HBM stacks (8 cores at once), delaying the
                # receipts that gate the first matmuls. Gate tile t's load on
                # tile t-6's first matmul: ~19us of steady-state slack, but
                # only a 6-tile burst at the head.
                if t >= 6:
                    bass_rust.add_dep_helper(
                        xdma.ins, b0_mms[t - 6].ins, True, "throttle prefetch"
                    )
                if t == 0:
                    # Issue the bulk weight slice after tile 0's input so it
                    # doesn't delay the first matmul, but early enough that
                    # its completion receipt lands before the b=4 round.
                    nc.sync.dma_start(wtrest[:], wt[:, 4 * MT :])

                acc = ppool.tile([MT, NT], F32, tag="acc", name=f"acc{t}")
                for b in range(KW):
                    mm = nc.tensor.matmul(
                        acc[:],
                        wslice(b),
                        xt[:, b : b + NT],
                        start=(b == 0),
                        stop=(b == KW - 1),
                    )
                    # Keep the tensor-engine stream in program order so the
                    # scheduler can't break the tight back-to-back pipeline.
                    if prev_mm is not None:
                        bass_rust.add_dep_helper(
                            mm.ins, prev_mm.ins, False, "pe stream order"
                        )
                    prev_mm = mm
                    if b == 0:
                        b0_mms.append(mm)

                ot = opool.tile([MT, NT], BF16, tag="ot", name=f"ot{t}")
                # Drain PSUM (fp32) to SBUF as bf16: the cast-on-copy runs
                # 2x faster on the DVE, the store DMA halves, and the final
                # tile's store receipt (which gates teardown) fires sooner.
                # Output rounding adds ~1e-3 relative error (gate is 2e-2);
                # the host converts back to fp32.
                if t < NTILES - 1:
                    nc.vector.tensor_copy(ot[:], acc[:])
                    nc.sync.dma_start(out[t * MT : (t + 1) * MT, :], ot[:, :])
                else:
                    # Last tile: pipeline the drain in two halves so the
                    # final store (whose completion receipt gates the
                    # teardown barrier) issues ~0.4us earlier.
                    h = 64
                    nc.vector.tensor_copy(ot[:h], acc[:h])
                    nc.sync.dma_start(out[t * MT : t * MT + h, :], ot[:h, :])
                    nc.vector.tensor_copy(ot[h:MT], acc[h:MT])
                    nc.sync.dma_start(
                        out[t * MT + h : (t + 1) * MT, :], ot[h:MT, :]
                    )
    nc.finalize()
    return nc


def _toeplitz_pack(weight: np.ndarray) -> np.ndarray:
    """Pack the 15 banded Toeplitz matrices T_b[r, m] = w[r-m, b] side by side."""
    wt = np.zeros((128, KW * MT), dtype=np.float32)
    r = np.arange(128)[:, None]
    m = np.arange(MT)[None, :]
    a = r - m  # tap index
    valid = (a >= 0) & (a < KH)
    av = np.where(valid, a, 0)
    for b in range(KW):
        wt[:, b * MT : (b + 1) * MT] = np.where(valid, weight[av, b], 0.0)
    return wt


def kernel(X: np.ndarray, weight: np.ndarray, bias: np.ndarray) -> np.ndarray:
    import ml_dtypes

    X = np.ascontiguousarray(X, dtype=np.float32)
    weight = np.ascontiguousarray(weight, dtype=np.float32)
    bias = np.asarray(bias, dtype=np.float32)

    Xr = X.astype(ml_dtypes.bfloat16)
    wt = _toeplitz_pack(weight).astype(ml_dtypes.bfloat16)

    in_maps = []
    for c in range(NCORES):
        xs = np.zeros((H_PAD, IN_COLS), dtype=ml_dtypes.bfloat16)
        c0 = c * COLS_PER_CORE
        c1 = min(c0 + IN_COLS, W)
        xs[:H, : c1 - c0] = Xr[:, c0:c1]
        in_maps.append({"x": xs, "wt": wt})

    nc = _build_program()
    res = run_bass_kernel_spmd(nc, in_maps, core_ids=list(range(NCORES)))
    global _last_results
    _last_results = res

    out = np.empty((OH, OW), dtype=np.float32)
    for c in range(NCORES):
        c0 = c * COLS_PER_CORE
        n = min(COLS_PER_CORE, OW - c0)
        out[:, c0 : c0 + n] = np.asarray(res.results[c]["out"][:OH, :n]).astype(
            np.float32
        )

    b0 = float(bias.reshape(-1)[0]) if bias.size else 0.0
    if b0 != 0.0:
        out += b0
    return out


# revision 26
# speedup vs baseline: 1.1932x; 1.1932x over previous
"""Trainium2 Bass kernel: 2D valid cross-correlation (4096x4096 image, 15x15 kernel).

Strategy: shard output COLUMNS across 8 NeuronCores (spatial data-parallel,
14-column halo overlap in the input slices; no device-to-device
communication). Each core computes all 4082 output rows for its 512 output
columns as 15 PSUM-accumulated bf16 matmuls per 114-row tile: for kernel
column b, the stationary operand is the 128x114 banded Toeplitz matrix
T_b[r, m] = w[r - m, b] (contraction over 128 input rows -> 114 output
rows) and the moving operand is the natural row-major X tile offset by b
columns in the free dimension. PSUM accumulation is fp32; the result is
stored as bf16 and widened to fp32 on the host.

bf16 weight loads use the hardware fast-weight-load path and pipeline under
the in-flight matmul stream (the PE pulls LDWEIGHTS ahead), so the 15
per-tile weight reloads are free and the stream runs at the 512-cycle
(~216ns) per-matmul floor. bf16 also halves input DMA traffic. X is padded
to uniform 128-row tiles on the host; input prefetch is 18 tiles deep and
PSUM banks rotate 8-deep. All input/output DMAs ride the sync HWDGE queue
and all PSUM->SBUF drains ride vector, leaving gpsimd/scalar idle (a gpsimd
SWDGE backlog costs ~5us of teardown drain; out-DMAs on the drain engine's
queue delay PSUM bank recycling). Dummy matmuls fill the ~3us window where
the first weight slice's DMA receipt gates the real stream, so the PE
clock-gate (1.2GHz cold / 2.4GHz warm) is already open when it starts.

Accuracy: inputs and the stored output are rounded to bf16 (~3e-3 relative
error combined vs the fp32 reference; the harness gate is 2e-2). PSUM
accumulation itself is exact fp32.
"""

import numpy as np

import bass_rust
import concourse.bass as bass
import concourse.mybir as mybir
import concourse.tile as tile
from concourse import bacc
from concourse.bass_utils import run_bass_kernel_spmd

H, W = 4096, 4096
KH, KW = 15, 15
OH, OW = H - KH + 1, W - KW + 1  # 4082 x 4082

NCORES = 8
COLS_PER_CORE = 512               # output cols per core (core 7: 498 valid)
IN_COLS = COLS_PER_CORE + KW - 1  # 526 input cols (with halo)

MT = 114                          # output rows per tile (K = MT + 14 = 128)
NT = COLS_PER_CORE                # 512 output cols = one fp32 PSUM bank
NTILES = 36                       # ceil(4082 / 114); 36*114 = 4104 rows
H_PAD = NTILES * MT + KH - 1      # 4118 padded input rows (uniform K=128)

F32 = mybir.dt.float32
BF16 = mybir.dt.bfloat16


def _build_program():
    nc = bacc.Bacc("TRN2", target_bir_lowering=False, debug=False)
    x = nc.dram_tensor("x", [H_PAD, IN_COLS], BF16, kind="ExternalInput").ap()
    wt = nc.dram_tensor("wt", [128, KW * MT], BF16, kind="ExternalInput").ap()
    out = nc.dram_tensor("out", [NTILES * MT, NT], BF16, kind="ExternalOutput").ap()

    with tile.TileContext(nc) as tc:
        with (
            tc.tile_pool(name="wpool", bufs=1) as wpool,
            tc.tile_pool(name="xpool", bufs=18) as xpool,
            tc.tile_pool(name="opool", bufs=8) as opool,
            tc.tile_pool(name="ppool", bufs=8, space="PSUM") as ppool,
        ):
            # Toeplitz pack, split into per-slice DMAs so the completion
            # receipts (~2.5us each) for the first kernel-column slices land
            # before the matmul stream reaches them. wt0 rides the otherwise
            # empty scalar queue: its receipt gates the first real matmul,
            # and on a shared queue it fires ~1.2us later.
            wt0 = wpool.tile([128, MT], BF16, tag="wt0")
            nc.scalar.dma_start(wt0[:], wt[:, :MT])
            wt12 = wpool.tile([128, 2 * MT], BF16, tag="wt12")
            nc.sync.dma_start(wt12[:], wt[:, MT : 3 * MT])
            wt3 = wpool.tile([128, MT], BF16, tag="wt3")
            nc.sync.dma_start(wt3[:], wt[:, 3 * MT : 4 * MT])
            wtrest = wpool.tile([128, (KW - 4) * MT], BF16, tag="wtr")

            def wslice(b):
                if b == 0:
                    return wt0[:, :]
                if b in (1, 2):
                    return wt12[:, (b - 1) * MT : b * MT]
                if b == 3:
                    return wt3[:, :]
                return wtrest[:, (b - 4) * MT : (b - 3) * MT]

            prev_mm = None

            # HAM pre-warm: the first real matmul is gated on the wt0 DMA
            # completion semaphore (~2.5us of receipt latency); run dummy
            # matmuls in that window so the PE clock-gate (4/8 when cold)
            # opens closer to the start of the real stream.
            dz = wpool.tile([128, NT], BF16, tag="dz")
            nc.vector.memset(dz[:], 0)
            dacc = ppool.tile([MT, NT], F32, tag="acc", name="dacc")
            for _ in range(7):
                mm = nc.tensor.matmul(
                    dacc[:], dz[:, :MT], dz[:], start=True, stop=True
                )
                if prev_mm is not None:
                    bass_rust.add_dep_helper(
                        mm.ins, prev_mm.ins, False, "pe stream order"
                    )
                prev_mm = mm
            b0_mms = []
            for t in range(NTILES):
                xt = xpool.tile([128, IN_COLS], BF16, tag="xt", name=f"xt{t}")
                xdma = nc.sync.dma_start(xt[:], x[t * MT : t * MT + 128, :])
                # Throttle the input prefetch: with all 18 pool slots free at
                # t=0, the sync engine would fire an 18-tile burst that
                # saturates the HBM stacks (8 cores at once), delaying the
                # receipts that gate the first matmuls. Gate tile t's load on
                # tile t-6's first matmul: ~19us of steady-state slack, but
                # only a 6-tile burst at the head.
                if t >= 6:
                    bass_rust.add_dep_helper(
                        xdma.ins, b0_mms[t - 6].ins, True, "throttle prefetch"
                    )
                if t == 0:
                    # Issue the bulk weight slice after tile 0's input so it
                    # doesn't delay the first matmul, but early enough that
                    # its completion receipt lands before the b=4 round.
                    nc.sync.dma_start(wtrest[:], wt[:, 4 * MT :])

                acc = ppool.tile([MT, NT], F32, tag="acc", name=f"acc{t}")
                for b in range(KW):
                    mm = nc.tensor.matmul(
                        acc[:],
                        wslice(b),
                        xt[:, b : b + NT],
                        start=(b == 0),
                        stop=(b == KW - 1),
                    )
                    # Keep the tensor-engine stream in program order so the
                    # scheduler can't break the tight back-to-back pipeline.
                    if prev_mm is not None:
                        bass_rust.add_dep_helper(
                            mm.ins, prev_mm.ins, False, "pe stream order"
                        )
                    prev_mm = mm
                    if b == 0:
                        b0_mms.append(mm)

                ot = opool.tile([MT, NT], BF16, tag="ot", name=f"ot{t}")
                # Drain PSUM (fp32) to SBUF as bf16: the cast-on-copy runs
                # 2x faster on the DVE, the store DMA halves, and the final
                # tile's store receipt (which gates teardown) fires sooner.
                # Output rounding adds ~1e-3 relative error (gate is 2e-2);
                # the host converts back to fp32.
                nc.vector.tensor_copy(ot[:], acc[:])
                nc.sync.dma_start(out[t * MT : (t + 1) * MT, :], ot[:, :])
    nc.finalize()
    return nc


def _toeplitz_pack(weight: np.ndarray) -> np.ndarray:
    """Pack the 15 banded Toeplitz matrices T_b[r, m] = w[r-m, b] side by side."""
    wt = np.zeros((128, KW * MT), dtype=np.float32)
    r = np.arange(128)[:, None]
    m = np.arange(MT)[None, :]
    a = r - m  # tap index
    valid = (a >= 0) & (a < KH)
    av = np.where(valid, a, 0)
    for b in range(KW):
        wt[:, b * MT : (b + 1) * MT] = np.where(valid, weight[av, b], 0.0)
    return wt


def kernel(X: np.ndarray, weight: np.ndarray, bias: np.ndarray) -> np.ndarray:
    import ml_dtypes

    X = np.ascontiguousarray(X, dtype=np.float32)
    weight = np.ascontiguousarray(weight, dtype=np.float32)
    bias = np.asarray(bias, dtype=np.float32)

    Xr = X.astype(ml_dtypes.bfloat16)
    wt = _toeplitz_pack(weight).astype(ml_dtypes.bfloat16)

    in_maps = []
    for c in range(NCORES):
        xs = np.zeros((H_PAD, IN_COLS), dtype=ml_dtypes.bfloat16)
        c0 = c * COLS_PER_CORE
        c1 = min(c0 + IN_COLS, W)
        xs[:H, : c1 - c0] = Xr[:, c0:c1]
        in_maps.append({"x": xs, "wt": wt})

    nc = _build_program()
    res = run_bass_kernel_spmd(nc, in_maps, core_ids=list(range(NCORES)))
    global _last_results
    _last_results = res

    out = np.empty((OH, OW), dtype=np.float32)
    for c in range(NCORES):
        c0 = c * COLS_PER_CORE
        n = min(COLS_PER_CORE, OW - c0)
        out[:, c0 : c0 + n] = np.asarray(res.results[c]["out"][:OH, :n]).astype(
            np.float32
        )

    b0 = float(bias.reshape(-1)[0]) if bias.size else 0.0
    if b0 != 0.0:
        out += b0
    return out
